# revision 52
# baseline (speedup 1.0000x reference)
"""Cross-view attention Trainium2 kernel.

Reference computation (per sample b):
    q = Wq @ x1 + bq            (D=64, N)      x1 = view1[b] as (C, N)
    k = Wk @ x2 + bk            (D, N)
    v = Wv @ x2 + bv            (C, N)
    S = q^T k                   (N, N)
    P = softmax(S, axis=-1)
    out = v @ P^T               (C, N)
    y = gamma * out + x1

Sharding: data-parallel over batch B=8 across the 8 NeuronCores (one
sample per core), no collectives.

Device algorithm (per core), v3:
  - Measured on this part, every ap=512 matmul costs ~254 ns regardless
    of dtype (fp8 DoubleRow included), so PE time == matmul instruction
    count.  The kernel minimizes instructions and keeps every engine's
    per-window work under the PE's.
  - Inputs arrive as f16 (host converts), so no on-chip input casts.
    Host also folds gamma into Wv and gamma*bv + view1 into the
    residual tensor v1p, eliminating all bias/scale matmuls.
  - Projections in fp16 straight from DMA; vT (keys on partitions) in
    bf16 for P.V.
  - S^T tiles (m keys x n queries) via K=64 fp16 matmuls, two per PE
    pass packed into disjoint row quadrants (tile_position).  exp on
    ScalarE in bf16 (logits are O(+-50), bf16 range is safe, no max
    subtraction needed).
  - P.V: bf16 matmuls accumulating over all key tiles in PSUM.  The
    softmax denominator l[n] is cheap: DVE pair/quad/oct-sums the exp
    tiles (bf16, 2x-rate) so only mt/8 ones[128,32] matmuls accumulate
    l (M=32: M=1 matmuls are measurably slower on hw).  fp8 DoubleRow
    was measured at the same ns/instruction as bf16 on this part and
    fp8e4m3 values fail the accuracy budget, so bf16 it is.
  - Epilogue (software-pipelined one window behind): r = 1/l via DVE
    reciprocal issued right after the last l matmul, broadcast to 128
    partitions by GpSimd (partition_broadcast, no PE matmul);
    y = acc*r + v1p with the multiply on DVE (PSUM read) and the add
    on GpSimd, overlapping the next window's S^T stream.
"""

import sys

if "/opt/trn_rl_repo" not in sys.path:
    sys.path.insert(0, "/opt/trn_rl_repo")

import numpy as np

B, C, H, W = 8, 512, 64, 64
D = C // 8            # 64
N = H * W             # 4096
CC = C // 128         # 4 chunks of the channel dim
NCORES = 8

_compiled = {}


def _build(n=N, repeat=1, nwin=512):
    from contextlib import ExitStack

    import concourse.mybir as mybir
    import concourse.tile as tile
    from concourse import bacc

    dt = mybir.dt
    f32, bf16, f16 = dt.float32, dt.bfloat16, dt.float16
    AF = mybir.ActivationFunctionType

    nwin = min(nwin, n)
    nch = n // nwin       # query windows
    mt = n // 128         # key tiles
    npairs = mt // 2

    nc = bacc.Bacc("TRN2", target_bir_lowering=False, debug=False)
    v1 = nc.dram_tensor("v1", [C, n], f16, kind="ExternalInput").ap()
    v1p = nc.dram_tensor("v1p", [C, n], f32, kind="ExternalInput").ap()
    v2 = nc.dram_tensor("v2", [C, n], f16, kind="ExternalInput").ap()
    wqT = nc.dram_tensor("wqT", [C, D], f32, kind="ExternalInput").ap()
    wkT = nc.dram_tensor("wkT", [C, D], f32, kind="ExternalInput").ap()
    wvT = nc.dram_tensor("wvT", [C, C], f32, kind="ExternalInput").ap()
    bq = nc.dram_tensor("bq", [1, D], f32, kind="ExternalInput").ap()
    bk = nc.dram_tensor("bk", [1, D], f32, kind="ExternalInput").ap()
    out = nc.dram_tensor("out", [C, n], f32, kind="ExternalOutput").ap()

    v1w = v1.rearrange("(cc p) n -> p cc n", p=128)
    v1pw = v1p.rearrange("(cc p) n -> p cc n", p=128)
    v2w = v2.rearrange("(cc p) n -> p cc n", p=128)
    outw = out.rearrange("(cc p) n -> p cc n", p=128)

    with tile.TileContext(nc) as tc, ExitStack() as top:
        consts = top.enter_context(tc.tile_pool(name="consts", bufs=1))

        # ---- constants ----
        wq_s = consts.tile([128, CC, D], f16, tag="wq")
        wk_s = consts.tile([128, CC, D], f16, tag="wk")
        wv_s = consts.tile([128, CC, C], f16, tag="wv")
        bqc_s = consts.tile([D, 1], f32, tag="bqc")   # ACT bias column
        bkc_s = consts.tile([D, 1], f32, tag="bkc")
        ones32 = consts.tile([128, 32], bf16, tag="ones32")  # accl lhsT (M=32)

        with ExitStack() as p0:
            wstp = p0.enter_context(tc.tile_pool(name="wst", bufs=1))
            stage_w = wstp.tile([128, CC, C], f32, tag="stage_w")
            nc.scalar.dma_start(stage_w[:, :, :D], wqT.rearrange("(cc p) d -> p cc d", p=128))
            nc.vector.tensor_copy(wq_s[:], stage_w[:, :, :D])
            nc.scalar.dma_start(stage_w[:, :, D : 2 * D], wkT.rearrange("(cc p) d -> p cc d", p=128))
            nc.vector.tensor_copy(wk_s[:], stage_w[:, :, D : 2 * D])
            nc.scalar.dma_start(stage_w[:], wvT.rearrange("(cc p) c -> p cc c", p=128))
            nc.vector.tensor_copy(wv_s[:], stage_w[:])

            nc.scalar.dma_start(bqc_s[:], bq.rearrange("o d -> d o"))
            nc.scalar.dma_start(bkc_s[:], bk.rearrange("o d -> d o"))

            ones_f32 = wstp.tile([128, 32], f32, tag="ones_f32")
            nc.vector.memset(ones_f32[:], 1.0)
            nc.vector.tensor_copy(ones32[:], ones_f32[:])

        def emit_rep(rep):
            with ExitStack() as rctx:
                per = rctx.enter_context(tc.tile_pool(name=f"persist{rep}", bufs=1))
                # qT/kT duplicated across both partition halves for the
                # row-packed (tile_position) S^T matmuls
                qT_s = per.tile([128, n], f16, tag="qT")
                kT_s = per.tile([128, n], f16, tag="kT")
                vT_s = per.tile([128, mt, C], bf16, tag="vT")

                # ================= phase 1: projections =================
                with ExitStack() as p1:
                    xst = p1.enter_context(tc.tile_pool(name=f"xst{rep}", bufs=3))
                    ps1 = p1.enter_context(
                        tc.tile_pool(name=f"ps1{rep}", bufs=2, space="PSUM")
                    )

                    for j in range(nch):
                        jw = slice(j * nwin, (j + 1) * nwin)
                        xs = xst.tile([128, CC, nwin], f16, tag="xs")
                        nc.sync.dma_start(xs[:, :2, :], v2w[:, :2, jw])
                        nc.gpsimd.dma_start(xs[:, 2:, :], v2w[:, 2:, jw])
                        xq = xst.tile([128, CC, nwin], f16, tag="xq")
                        nc.sync.dma_start(xq[:, :2, :], v1w[:, :2, jw])
                        nc.gpsimd.dma_start(xq[:, 2:, :], v1w[:, 2:, jw])
                        ps = ps1.tile([64, nwin], f32, tag="psqk")
                        for cc in range(CC):
                            nc.tensor.matmul(
                                ps[:],
                                wk_s[:, cc, :],
                                xs[:, cc, :],
                                start=(cc == 0),
                                stop=(cc == CC - 1),
                            )
                        nc.scalar.activation(
                            kT_s[:64, jw], ps[:], AF.Identity, bias=bkc_s[:]
                        )
                        nc.sync.dma_start(kT_s[64:128, jw], kT_s[:64, jw])
                        psq = ps1.tile([64, nwin], f32, tag="psq")
                        for cc in range(CC):
                            nc.tensor.matmul(
                                psq[:],
                                wq_s[:, cc, :],
                                xq[:, cc, :],
                                start=(cc == 0),
                                stop=(cc == CC - 1),
                            )
                        nc.scalar.activation(
                            qT_s[:64, jw], psq[:], AF.Identity, bias=bqc_s[:]
                        )
                        nc.sync.dma_start(qT_s[64:128, jw], qT_s[:64, jw])
                        for mi in range(nwin // 128):
                            m = j * (nwin // 128) + mi
                            miw = slice(mi * 128, (mi + 1) * 128)
                            psv = ps1.tile([128, C], f32, tag="psv")
                            for cc in range(CC):
                                nc.tensor.matmul(
                                    psv[:],
                                    xs[:, cc, miw],
                                    wv_s[:, cc, :],
                                    start=(cc == 0),
                                    stop=(cc == CC - 1),
                                )
                            nc.scalar.activation(vT_s[:, m, :], psv[:], AF.Copy)


                # ================= phase 2: attention =================
                with ExitStack() as p2:
                    psS = p2.enter_context(
                        tc.tile_pool(name=f"psS{rep}", bufs=3, space="PSUM")
                    )
                    psA = p2.enter_context(
                        tc.tile_pool(name=f"psA{rep}", bufs=1, space="PSUM")
                    )
                    psL = p2.enter_context(
                        tc.tile_pool(name=f"psL{rep}", bufs=1, space="PSUM")
                    )
                    expp = p2.enter_context(tc.tile_pool(name=f"expp{rep}", bufs=10))
                    psump = p2.enter_context(tc.tile_pool(name=f"pairs{rep}", bufs=4))
                    smalls = p2.enter_context(tc.tile_pool(name=f"smalls{rep}", bufs=2))
                    rbp = p2.enter_context(tc.tile_pool(name=f"rbp{rep}", bufs=2))
                    resp = p2.enter_context(tc.tile_pool(name=f"resp{rep}", bufs=3))
                    outp_sb = p2.enter_context(tc.tile_pool(name=f"outp{rep}", bufs=3))

                    def emit_recip(accl):
                        # 1/l + broadcast, emitted right after the last accl
                        # matmul so it overlaps the window's tail P.V work
                        r_sb = smalls.tile([1, nwin], f32, tag="r", name="r_sb")
                        nc.vector.reciprocal(r_sb[:], accl[:1, :])
                        rb_sb = rbp.tile([128, nwin], f32, tag="rb", name="rb_sb")
                        nc.gpsimd.partition_broadcast(rb_sb[:], r_sb[:])
                        return rb_sb

                    def emit_epilogue(j, accs, rb_sb):
                        # y = acc * (1/l) + v1p   (gamma folded into Wv)
                        jw = slice(j * nwin, (j + 1) * nwin)
                        for ct in range(CC):
                            v1c = resp.tile([128, nwin], f32, tag="v1c", name="v1c")
                            nc.sync.dma_start(v1c[:], v1pw[:, ct, jw])
                            t_sb = outp_sb.tile([128, nwin], f32, tag="t", name="t_sb")
                            nc.vector.tensor_mul(t_sb[:], accs[ct][:], rb_sb[:])
                            o_sb = outp_sb.tile([128, nwin], f32, tag="o", name="o_sb")
                            nc.gpsimd.tensor_add(o_sb[:], t_sb[:], v1c[:])
                            nc.gpsimd.dma_start(outw[:, ct, jw], o_sb[:])

                    npairs_ = npairs
                    pend_epi = None
                    for j in range(nch):
                        jw = slice(j * nwin, (j + 1) * nwin)
                        # one PSUM tile (= one full bank) per output c-chunk:
                        # accumulation groups must not share a bank
                        accs = [
                            psA.tile([128, nwin], f32, tag=f"acc{ct}", name=f"acc{ct}")
                            for ct in range(CC)
                        ]
                        accl = psL.tile([32, nwin], f32, tag="accl")
                        # software pipeline: issue S^T/exp of pair i+1 before
                        # the P.V matmuls of pair i, so ScalarE's exp overlaps
                        # TensorE's P.V; the previous chunk's epilogue is
                        # emitted after this chunk's first S^T pair
                        prev = None
                        prev2 = None
                        quad = None
                        oct_ = None
                        nacc = 0
                        pair_q = []
                        for m2 in range(npairs_ + 2):
                            cur = None
                            if m2 < npairs_:
                                sts = []
                                for half in (0, 1):
                                    m = 2 * m2 + half
                                    mw = slice(m * 128, (m + 1) * 128)
                                    hp = slice(64 * half, 64 * half + 64)
                                    st = psS.tile([128, nwin], f32, tag="st", name="st")
                                    nc.tensor.matmul(
                                        st[:],
                                        kT_s[hp, mw],
                                        qT_s[hp, jw],
                                        start=True,
                                        stop=True,
                                        tile_position=(64 * half, 0),
                                    )
                                    sts.append(st)
                                exs = []
                                for half in (0, 1):
                                    ex = expp.tile([128, nwin], bf16, tag="ex", name="ex")
                                    nc.scalar.activation(ex[:], sts[half][:], AF.Exp)
                                    exs.append(ex)
                                # pair-sum on DVE, then quad/oct sums: 8x
                                # fewer l matmuls
                                psum2 = psump.tile(
                                    [128, nwin], bf16, tag="ps2", name="ps2"
                                )
                                with nc.allow_low_precision(reason="l pair-sum"):
                                    nc.vector.tensor_add(
                                        psum2[:], exs[0][:], exs[1][:]
                                    )
                                cur = (exs, psum2)
                            if m2 == 1 and pend_epi is not None:
                                emit_epilogue(*pend_epi)
                                pend_epi = None
                            if m2 > 1:
                                exs, psum2 = prev2
                                for half in (0, 1):
                                    m = 2 * (m2 - 2) + half
                                    ex = exs[half]
                                    for ct in range(CC):
                                        nc.tensor.matmul(
                                            accs[ct][:],
                                            vT_s[:, m, ct * 128 : (ct + 1) * 128],
                                            ex[:],
                                            start=(m == 0),
                                            stop=(m == mt - 1),
                                        )
                                if quad is None:
                                    quad = psum2
                                else:
                                    psum4 = psump.tile(
                                        [128, nwin], bf16, tag="ps4", name="ps4"
                                    )
                                    with nc.allow_low_precision(reason="l quad-sum"):
                                        nc.vector.tensor_add(
                                            psum4[:], quad[:], psum2[:]
                                        )
                                    quad = None
                                    if oct_ is None:
                                        oct_ = psum4
                                    else:
                                        psum8 = psump.tile(
                                            [128, nwin], bf16, tag="ps8", name="ps8"
                                        )
                                        with nc.allow_low_precision(reason="l oct-sum"):
                                            nc.vector.tensor_add(
                                                psum8[:], oct_[:], psum4[:]
                                            )
                                        oct_ = None
                                        nacc += 1
                                        nc.tensor.matmul(
                                            accl[:],
                                            ones32[:],
                                            psum8[:],
                                            start=(nacc == 1),
                                            stop=(nacc == npairs_ // 4),
                                        )
                                        if nacc == npairs_ // 4:
                                            rb_sb = emit_recip(accl)
                            prev2 = prev
                            prev = cur
                        pend_epi = (j, accs, rb_sb)
                    emit_epilogue(*pend_epi)

        if repeat == 1:
            emit_rep(0)
        else:
            with tc.For_i(0, repeat, 1):
                emit_rep(0)

    nc.compile()
    return nc


def _get_nc(n=N, repeat=1):
    key = (n, repeat)
    if key not in _compiled:
        _compiled[key] = _build(n=n, repeat=repeat)
    return _compiled[key]


def _run_res(nc, view1, view2, Wq, bq, Wk, bk, Wv, bv, gamma, n=N, **spmd_kwargs):
    from concourse.bass_utils import run_bass_kernel_spmd

    b = view1.shape[0]
    f = np.ascontiguousarray
    gamma_f = np.float32(np.asarray(gamma).reshape(-1)[0])
    bv_col = np.asarray(bv, np.float32).reshape(C, 1)
    com = {
        "wqT": f(Wq.T.astype(np.float32)),
        "wkT": f(Wk.T.astype(np.float32)),
        # gamma folded into the value projection
        "wvT": f((Wv.T * gamma_f).astype(np.float32)),
        "bq": f(bq.reshape(1, D).astype(np.float32)),
        "bk": f(bk.reshape(1, D).astype(np.float32)),
    }
    in_maps = []
    for i in range(NCORES):
        bi = min(i, b - 1)  # replicate last sample if b < NCORES
        x1 = view1[bi].reshape(C, n).astype(np.float32)
        x2 = view2[bi].reshape(C, n).astype(np.float32)
        in_maps.append(
            {
                "v1": f(x1.astype(np.float16)),
                "v1p": f(x1 + gamma_f * bv_col),
                "v2": f(x2.astype(np.float16)),
                **com,
            }
        )
    res = run_bass_kernel_spmd(nc, in_maps, list(range(NCORES)), **spmd_kwargs)
    outs = [res.results[i]["out"] for i in range(b)]
    return np.stack(outs, axis=0), res


def _run(*args, **kwargs):
    return _run_res(*args, **kwargs)[0]


def kernel(view1, view2, Wq, bq, Wk, bk, Wv, bv, gamma):
    view1 = np.asarray(view1)
    b, c, h, w = view1.shape
    n = h * w
    nc = _get_nc(n=n, repeat=1)
    out = _run(
        nc,
        np.asarray(view1),
        np.asarray(view2),
        np.asarray(Wq),
        np.asarray(bq),
        np.asarray(Wk),
        np.asarray(bk),
        np.asarray(Wv),
        np.asarray(bv),
        np.asarray(gamma),
        n=n,
    )
    return out.reshape(b, c, h, w).astype(np.float32)
